# revision 23
# baseline (speedup 1.0000x reference)
"""Cross-attention (cosine/l2-normalized, biased softmax) on 8 TRN2 NeuronCores.

Sharding: core c handles batch b = c//2 and head group g = c%2 (8 of 16 heads,
i.e. a 512-wide slice of the QKV projections / Wo rows).  Each core computes a
partial output (its heads' contribution through Wo); the host sums the two
partials per batch and adds bo.

All tensors are kept transposed on chip (channels on partitions):
  qT/kT = (Wx)^T computed as lhsT=W, rhs=x^T; v in natural [j, ch] layout via
  lhsT=y^T.  Scores are computed transposed S^T[j, i] (lhsT = kn^T slice,
  rhs = qn^T slice, K = 64), softmax runs unnormalized as exp(S)*exp(bias)
  with the denominator obtained from an extra all-ones column appended to V,
  and the division happens after the PV matmul (partition_broadcast + mul).
L2-norm denominators use a block-diagonal selector matmul (K=128) for the
per-head sum of squares and exp(-0.5*ln(x)) on the scalar engine (the DVE
reciprocal op is ~5x an ACT pass; ACT Rsqrt is disallowed).

Matmuls run in float32r (rounded fp32, ~1.2e-4 relative); with
KERNEL_ATTN_DT=bf16 the attention matmuls (scores / PV) and the probability
tensor run in bf16 instead (f32r K=64 matmuls run at ~3.4 cyc/row on HW,
bf16 at ~1.5).
"""

import os
import numpy as np
import ml_dtypes

import concourse.bass as bass
import concourse.tile as tile
from concourse import bacc, mybir
from concourse.bass_utils import run_bass_kernel_spmd

F32 = mybir.dt.float32
F32R = mybir.dt.float32r
BF16 = mybir.dt.bfloat16

B, Lq, Ly, C = 4, 1024, 2048, 1024
H_TOT, D = 16, 64
HL = 8           # heads per core
CHL = HL * D     # 512 channels per core
TCH = CHL // 128  # 4 channel tiles (2 heads each)
KT = C // 128     # 8 contraction tiles for the projections
NJT = Ly // 128   # 16 j tiles
NIC = Lq // 512   # 2 i chunks
N_CORES = 8
MAX_SCALE_MUL = float(np.log(100.0))

# attention dtype: "f32" (f32r matmuls, fp32 probs) or "bf16"
ATTN_DT = os.environ.get("KERNEL_ATTN_DT", "bf16")

LAST_EXEC_NS = None
_COMPILED = {}
Exp = mybir.ActivationFunctionType.Exp
Ln = mybir.ActivationFunctionType.Ln


def _build(attn_dt: str):
    bf = attn_dt != "f32"
    AD = BF16 if bf else F32

    def mmcast(ap):
        # matmul operand dtype for the attention matmuls
        return ap if bf else ap.bitcast(F32R)

    nc = bacc.Bacc("TRN2", target_bir_lowering=False, debug=False,
                   num_devices=N_CORES)

    xT_ap = nc.dram_tensor("xT", [C, Lq], F32, kind="ExternalInput").ap()
    yT_ap = nc.dram_tensor("yT", [C, Ly], F32, kind="ExternalInput").ap()
    wq_ap = nc.dram_tensor("wq", [C, CHL], F32, kind="ExternalInput").ap()
    wk_ap = nc.dram_tensor("wk", [C, CHL], F32, kind="ExternalInput").ap()
    wv_ap = nc.dram_tensor("wv", [C, CHL], F32, kind="ExternalInput").ap()
    wo_ap = nc.dram_tensor("wo", [CHL, C], F32, kind="ExternalInput").ap()
    bq_ap = nc.dram_tensor("bq4", [128, TCH], F32, kind="ExternalInput").ap()
    is2_ap = nc.dram_tensor("invs2", [128, TCH], F32, kind="ExternalInput").ap()
    eb_ap = nc.dram_tensor("ebT", [Ly, Lq], AD, kind="ExternalInput").ap()
    out_ap = nc.dram_tensor("outT", [C, Lq], F32, kind="ExternalOutput").ap()

    xT_d = xT_ap.rearrange("(k p) i -> p k i", p=128)
    yT_d = yT_ap.rearrange("(k p) j -> p k j", p=128)
    wq_d = wq_ap.rearrange("(k p) n -> p k n", p=128)
    wk_d = wk_ap.rearrange("(k p) n -> p k n", p=128)
    wv_d = wv_ap.rearrange("(k p) n -> p k n", p=128)
    wo_d = wo_ap.rearrange("(k p) n -> p k n", p=128)
    eb_d = eb_ap.rearrange("(jt p) i -> p jt i", p=128)
    out_d = out_ap.rearrange("(ct p) i -> p ct i", p=128)

    with tile.TileContext(nc) as tc:
        with tc.tile_pool(name="persist", bufs=1) as pers, \
             tc.tile_pool(name="qn_p", bufs=1) as qn_p, \
             tc.tile_pool(name="kn_p", bufs=1) as kn_p, \
             tc.tile_pool(name="v_p", bufs=1) as v_p:

            # block-diagonal parity selector: sel.T @ sq sums each 64-row
            # head block and replicates the sums over that block's rows
            sel_f = pers.tile([128, 128], F32)
            nc.gpsimd.memset(sel_f[:], 0.0)
            nc.gpsimd.memset(sel_f[0:64, 0:64], 1.0)
            nc.gpsimd.memset(sel_f[64:128, 64:128], 1.0)
            sel_r = pers.tile([128, 128], F32)
            nc.vector.tensor_copy(sel_r[:].bitcast(F32R), sel_f[:])
            ones_a = pers.tile([128, 8], AD)
            nc.gpsimd.memset(ones_a[:], 1.0)
            bq_sb = pers.tile([128, TCH], F32)
            nc.sync.dma_start(bq_sb[:], bq_ap[:])
            is2_sb = pers.tile([128, TCH], F32)
            nc.sync.dma_start(is2_sb[:], is2_ap[:])

            qn_sb = qn_p.tile([128, TCH, Lq], AD)     # qn^T
            kn_sb = kn_p.tile([128, TCH, Ly], AD)     # kn^T
            v_sb = v_p.tile([128, NJT, HL * 65], AD)  # v (+ ones col per head)

            # yT/wv space is reserved up front; their DMAs are emitted after
            # the q-phase loads so x/wq win the DMA queues first.
            with tc.tile_pool(name="yT_p", bufs=1) as yT_p, \
                 tc.tile_pool(name="wv_p", bufs=1) as wv_p:
                yT_sb = yT_p.tile([128, KT, Ly], F32)
                wv_sb = wv_p.tile([128, KT, CHL], F32)

                # -------------- Q projection + l2norm(+scale) --------------
                with tc.tile_pool(name="xT_p", bufs=1) as xT_p, \
                     tc.tile_pool(name="wq_p", bufs=1) as wq_p, \
                     tc.tile_pool(name="qtmp", bufs=2) as qtmp, \
                     tc.tile_pool(name="qnrm", bufs=1) as qnrm, \
                     tc.tile_pool(name="psq", bufs=4, space="PSUM") as psq, \
                     tc.tile_pool(name="psqs", bufs=2, space="PSUM") as psqs:
                    xT_sb = xT_p.tile([128, KT, Lq], F32)
                    for k in range(KT):
                        nc.sync.dma_start(xT_sb[:, k, :].bitcast(F32R),
                                          xT_d[:, k, :].bitcast(F32R))
                    wq_sb = wq_p.tile([128, KT, CHL], F32)
                    nc.sync.dma_start(wq_sb[:].bitcast(F32R),
                                      wq_d[:].bitcast(F32R))
                    for k in range(KT):
                        nc.sync.dma_start(yT_sb[:, k, :].bitcast(F32R),
                                          yT_d[:, k, :].bitcast(F32R))
                    nc.sync.dma_start(wv_sb[:].bitcast(F32R),
                                      wv_d[:].bitcast(F32R))

                    for t in range(TCH):
                        qraw = qtmp.tile([128, Lq], F32, tag="qraw")
                        for ic in range(NIC):
                            ps = psq.tile([128, 512], F32)
                            for k in range(KT):
                                nc.tensor.matmul(
                                    ps[:],
                                    wq_sb[:, k, t * 128:(t + 1) * 128].bitcast(F32R),
                                    xT_sb[:, k, ic * 512:(ic + 1) * 512].bitcast(F32R),
                                    start=(k == 0), stop=(k == KT - 1))
                            nc.vector.tensor_scalar_add(
                                qraw[:, ic * 512:(ic + 1) * 512], ps[:],
                                bq_sb[:, t:t + 1])
                        sq = qnrm.tile([128, Lq], F32, tag="sq")
                        nc.vector.tensor_mul(sq[:].bitcast(F32R), qraw[:], qraw[:])
                        rs = qnrm.tile([128, Lq], F32, tag="rs")
                        for ic in range(NIC):
                            ssq = psqs.tile([128, 512], F32)
                            nc.tensor.matmul(
                                ssq[:], sel_r[:].bitcast(F32R),
                                sq[:, ic * 512:(ic + 1) * 512].bitcast(F32R),
                                start=True, stop=True)
                            # s_h/|q| = exp(-0.5*ln(sumsq/s_h^2))
                            nc.scalar.activation(
                                rs[:, ic * 512:(ic + 1) * 512], ssq[:], Ln,
                                scale=is2_sb[:, t:t + 1])
                        nc.scalar.activation(rs[:], rs[:], Exp, scale=-0.5)
                        nc.vector.tensor_mul(mmcast(qn_sb[:, t, :]),
                                             qraw[:], rs[:])

                # ------- K projection + l2norm, V proj interleaved -------
                # v matmuls are emitted between k tiles so the PE stays busy
                # while the k-norm DVE/ACT chain drains.
                with tc.tile_pool(name="wk_p", bufs=1) as wk_p, \
                     tc.tile_pool(name="ktmp", bufs=1) as ktmp, \
                     tc.tile_pool(name="psk", bufs=2, space="PSUM") as psk, \
                     tc.tile_pool(name="psv", bufs=2, space="PSUM") as psv, \
                     tc.tile_pool(name="psks", bufs=2, space="PSUM") as psks:
                    wk_sb = wk_p.tile([128, KT, CHL], F32)
                    nc.sync.dma_start(wk_sb[:].bitcast(F32R),
                                      wk_d[:].bitcast(F32R))
                    for t in range(TCH):
                        kraw = ktmp.tile([128, Ly], F32, tag="kraw", bufs=2)
                        for jc in range(Ly // 512):
                            ps = psk.tile([128, 512], F32, tag="kps")
                            for k in range(KT):
                                nc.tensor.matmul(
                                    ps[:],
                                    wk_sb[:, k, t * 128:(t + 1) * 128].bitcast(F32R),
                                    yT_sb[:, k, jc * 512:(jc + 1) * 512].bitcast(F32R),
                                    start=(k == 0), stop=(k == KT - 1))
                            nc.vector.tensor_copy(
                                kraw[:, jc * 512:(jc + 1) * 512], ps[:])
                        sqk = ktmp.tile([128, Ly], F32, tag="sqk")
                        nc.vector.tensor_mul(sqk[:].bitcast(F32R), kraw[:], kraw[:])
                        rsk = ktmp.tile([128, Ly], F32, tag="rsk")
                        for jc in range(Ly // 512):
                            ssq = psks.tile([128, 512], F32)
                            nc.tensor.matmul(
                                ssq[:], sel_r[:].bitcast(F32R),
                                sqk[:, jc * 512:(jc + 1) * 512].bitcast(F32R),
                                start=True, stop=True)
                            nc.scalar.activation(
                                rsk[:, jc * 512:(jc + 1) * 512], ssq[:], Ln)
                        nc.scalar.activation(rsk[:], rsk[:], Exp, scale=-0.5)
                        nc.vector.tensor_mul(mmcast(kn_sb[:, t, :]),
                                             kraw[:], rsk[:])

                        # v projection for jt = 4t .. 4t+3
                        for jt in range(4 * t, 4 * t + 4):
                            ps = psv.tile([128, 512], F32, tag="vps")
                            for k in range(KT):
                                nc.tensor.matmul(
                                    ps[:],
                                    yT_sb[:, k, jt * 128:(jt + 1) * 128].bitcast(F32R),
                                    wv_sb[:, k, :].bitcast(F32R),
                                    start=(k == 0), stop=(k == KT - 1))
                            vslot = v_sb[:, jt, :].rearrange(
                                "p (h e) -> p h e", e=65)
                            nc.vector.tensor_copy(
                                mmcast(vslot[:, :, 0:64]),
                                ps[:].rearrange("p (h e) -> p h e", e=64))
                            nc.vector.tensor_copy(mmcast(vslot[:, :, 64:65]),
                                                  ones_a[:, 0:HL].unsqueeze(2))

            # ---------------- attention + output projection ----------------
            with tc.tile_pool(name="wo_p", bufs=1) as wo_p, \
                 tc.tile_pool(name="oT_p", bufs=1) as oT_p, \
                 tc.tile_pool(name="eb_p", bufs=17) as eb_p, \
                 tc.tile_pool(name="pp_p", bufs=3) as pp_p, \
                 tc.tile_pool(name="nrm_p", bufs=2) as nrm_p, \
                 tc.tile_pool(name="ost_p", bufs=2) as ost_p, \
                 tc.tile_pool(name="pss", bufs=2, space="PSUM") as pss, \
                 tc.tile_pool(name="pso", bufs=3, space="PSUM") as pso, \
                 tc.tile_pool(name="psf", bufs=1, space="PSUM") as psf:
                wo_sb = wo_p.tile([128, TCH, C], F32)
                nc.sync.dma_start(wo_sb[:].bitcast(F32R), wo_d[:].bitcast(F32R))
                oT_sb = oT_p.tile([128, TCH, Lq], F32)

                for ic in range(NIC):
                    ebs = []
                    for jt in range(NJT):
                        if bf:
                            # duplicated halves so the prob multiply is a
                            # plain step-1 2D op (DVE 2x bf16 mode)
                            ebt = eb_p.tile([128, 1024], AD, tag="eb",
                                            name=f"eb{ic}_{jt}")
                            nc.sync.dma_start(
                                ebt[:, 0:512],
                                eb_d[:, jt, ic * 512:(ic + 1) * 512])
                            nc.sync.dma_start(
                                ebt[:, 512:1024],
                                eb_d[:, jt, ic * 512:(ic + 1) * 512])
                        else:
                            ebt = eb_p.tile([128, 512], AD, tag="eb",
                                            name=f"eb{ic}_{jt}")
                            nc.sync.dma_start(
                                ebt[:], eb_d[:, jt, ic * 512:(ic + 1) * 512])
                        ebs.append(ebt)
                    oas = []
                    srows = []
                    for hp in range(TCH):
                        opsums = [pso.tile([65, 512], F32, tag="opsum",
                                           name=f"opsum{_p}")
                                  for _p in range(2)]
                        for jt in range(NJT):
                            s2 = pss.tile([128, 1024], F32, tag="s")
                            with tc.tile_critical():
                                # adjacent in the PE stream -> the two K=64
                                # row groups execute concurrently
                                for p in range(2):
                                    nc.tensor.matmul(
                                        s2[:, p * 512:(p + 1) * 512],
                                        mmcast(kn_sb[p * 64:p * 64 + 64, hp,
                                                     jt * 128:(jt + 1) * 128]),
                                        mmcast(qn_sb[p * 64:p * 64 + 64, hp,
                                                     ic * 512:(ic + 1) * 512]),
                                        start=True, stop=True,
                                        tile_position=(p * 64, 0) if bf else None)
                            p0 = pp_p.tile([128, 1024], AD, tag="p0")
                            nc.scalar.activation(p0[:], s2[:], Exp)
                            pt = pp_p.tile([128, 1024], AD, tag="pt")
                            if bf:
                                nc.vector.tensor_mul(pt[:], p0[:], ebs[jt][:])
                            else:
                                nc.vector.tensor_mul(
                                    mmcast(pt[:].rearrange(
                                        "q (p i) -> q p i", p=2)),
                                    p0[:].rearrange("q (p i) -> q p i", p=2),
                                    ebs[jt][:].unsqueeze(1).broadcast_to(
                                        [128, 2, 512]))
                            for p in range(2):
                                h = 2 * hp + p
                                nc.tensor.matmul(
                                    opsums[p][:],
                                    mmcast(v_sb[:, jt, h * 65:h * 65 + 65]),
                                    mmcast(pt[:, p * 512:(p + 1) * 512]),
                                    start=(jt == 0), stop=(jt == NJT - 1))
                        for p in range(2):
                            oa = nrm_p.tile([65, 512], F32, tag="oa", bufs=9,
                                            name=f"oa{hp}_{p}")
                            nc.vector.tensor_copy(oa[:], opsums[p][:])
                            srow = nrm_p.tile([1, 512], F32, tag="srow",
                                              bufs=9, name=f"srow{hp}_{p}")
                            nc.vector.tensor_copy(srow[:], opsums[p][64:65, :])
                            oas.append(oa)
                            srows.append(srow)
                    # deferred denominators: 1/x = exp(-ln(x)); the Ln calls
                    # run adjacent, then all Exp, so the ACT spline tables
                    # load only ~2x per i-chunk instead of per head
                    with tc.tile_critical():
                        for srow in srows:
                            nc.scalar.activation(srow[:], srow[:], Ln)
                        for srow in srows:
                            nc.scalar.activation(srow[:], srow[:], Exp,
                                                 scale=-1.0)
                    for i8 in range(2 * TCH):
                        hp, p = i8 // 2, i8 % 2
                        bct = nrm_p.tile([64, 512], F32, tag="bct",
                                         name=f"bct{i8}")
                        nc.gpsimd.partition_broadcast(bct[:], srows[i8][0:1, :])
                        nc.vector.tensor_mul(
                            oT_sb[p * 64:p * 64 + 64, hp,
                                  ic * 512:(ic + 1) * 512].bitcast(F32R),
                            oas[i8][0:64, :], bct[:])

                    # output projection for this i-chunk
                    for ct in range(C // 128):
                        ps = psf.tile([128, 512], F32, tag="fout")
                        for k in range(TCH):
                            nc.tensor.matmul(
                                ps[:],
                                wo_sb[:, k, ct * 128:(ct + 1) * 128].bitcast(F32R),
                                oT_sb[:, k, ic * 512:(ic + 1) * 512].bitcast(F32R),
                                start=(k == 0), stop=(k == TCH - 1))
                        st = ost_p.tile([128, 512], F32, tag="ost")
                        nc.vector.tensor_copy(st[:], ps[:])
                        nc.sync.dma_start(
                            out_d[:, ct, ic * 512:(ic + 1) * 512], st[:])

    nc.compile()
    return nc


def _get_compiled(attn_dt: str):
    if attn_dt not in _COMPILED:
        _COMPILED[attn_dt] = _build(attn_dt)
    return _COMPILED[attn_dt]


def kernel(x, y, attn_bias, Wq, bq, Wk, Wv, Wo, bo, scale_mul_log):
    global LAST_EXEC_NS
    attn_dt = ATTN_DT
    x = np.asarray(x, dtype=np.float32)
    y = np.asarray(y, dtype=np.float32)
    attn_bias = np.asarray(attn_bias, dtype=np.float32)
    Wq = np.asarray(Wq, dtype=np.float32)
    bq = np.asarray(bq, dtype=np.float32)
    Wk = np.asarray(Wk, dtype=np.float32)
    Wv = np.asarray(Wv, dtype=np.float32)
    Wo = np.asarray(Wo, dtype=np.float32)
    bo = np.asarray(bo, dtype=np.float32)
    scale_mul_log = np.asarray(scale_mul_log, dtype=np.float32)

    nc = _get_compiled(attn_dt)

    scale = np.exp(np.minimum(scale_mul_log.reshape(H_TOT), MAX_SCALE_MUL))
    ebT = np.exp(attn_bias.T)
    ebT = np.ascontiguousarray(
        ebT.astype(np.float32 if attn_dt == "f32" else ml_dtypes.bfloat16))

    xTs = [np.ascontiguousarray(x[b].T) for b in range(B)]
    yTs = [np.ascontiguousarray(y[b].T) for b in range(B)]

    in_maps = []
    for c in range(N_CORES):
        b, g = c // 2, c % 2
        sl = slice(g * CHL, (g + 1) * CHL)
        s_loc = scale[g * HL:(g + 1) * HL]       # 8 local heads
        inv2 = 1.0 / (s_loc * s_loc)
        # invs2[p, t] = 1/s^2 of head (2t + p//64)
        invs2 = np.empty((128, TCH), dtype=np.float32)
        for t in range(TCH):
            invs2[0:64, t] = inv2[2 * t]
            invs2[64:128, t] = inv2[2 * t + 1]
        bq4 = np.ascontiguousarray(bq[sl].reshape(TCH, 128).T)
        in_maps.append({
            "xT": xTs[b],
            "yT": yTs[b],
            "wq": np.ascontiguousarray(Wq[:, sl]),
            "wk": np.ascontiguousarray(Wk[:, sl]),
            "wv": np.ascontiguousarray(Wv[:, sl]),
            "wo": np.ascontiguousarray(Wo[sl, :]),
            "bq4": bq4,
            "invs2": invs2,
            "ebT": ebT,
        })

    trace = os.environ.get("KERNEL_TRACE", "0") == "1"
    res = run_bass_kernel_spmd(nc, in_maps, core_ids=list(range(N_CORES)),
                               trace=trace)
    LAST_EXEC_NS = res.exec_time_ns

    out = np.empty((B, Lq, C), dtype=np.float32)
    for b in range(B):
        out[b] = res.results[2 * b]["outT"].T
        out[b] += res.results[2 * b + 1]["outT"].T
    out += bo
    return out


# revision 24
# speedup vs baseline: 1.4079x; 1.4079x over previous
"""Cross-attention (cosine/l2-normalized, biased softmax) on 8 TRN2 NeuronCores.

Sharding: core c handles batch b = c//2 and head group g = c%2 (8 of 16 heads,
i.e. a 512-wide slice of the QKV projections / Wo rows).  Each core computes a
partial output (its heads' contribution through Wo); the host sums the two
partials per batch and adds bo.

All tensors are kept transposed on chip (channels on partitions):
  qT/kT = (Wx)^T computed as lhsT=W, rhs=x^T; v in natural [j, ch] layout via
  lhsT=y^T.  Scores are computed transposed S^T[j, i] (lhsT = kn^T slice,
  rhs = qn^T slice, K = 64), softmax runs unnormalized as exp(S)*exp(bias)
  with the denominator obtained from an extra all-ones column appended to V,
  and the division happens after the PV matmul (partition_broadcast + mul).
L2-norm denominators use a block-diagonal selector matmul (K=128) for the
per-head sum of squares and exp(-0.5*ln(x)) on the scalar engine (the DVE
reciprocal op is ~5x an ACT pass; ACT Rsqrt is disallowed).

Matmuls run in float32r (rounded fp32, ~1.2e-4 relative); with
KERNEL_ATTN_DT=bf16 the attention matmuls (scores / PV) and the probability
tensor run in bf16 instead (f32r K=64 matmuls run at ~3.4 cyc/row on HW,
bf16 at ~1.5).
"""

import os
import numpy as np
import ml_dtypes

import concourse.bass as bass
import concourse.tile as tile
from concourse import bacc, mybir
from concourse.bass_utils import run_bass_kernel_spmd

F32 = mybir.dt.float32
F32R = mybir.dt.float32r
BF16 = mybir.dt.bfloat16

B, Lq, Ly, C = 4, 1024, 2048, 1024
H_TOT, D = 16, 64
HL = 8           # heads per core
CHL = HL * D     # 512 channels per core
TCH = CHL // 128  # 4 channel tiles (2 heads each)
KT = C // 128     # 8 contraction tiles for the projections
NJT = Ly // 128   # 16 j tiles
NIC = Lq // 512   # 2 i chunks
N_CORES = 8
MAX_SCALE_MUL = float(np.log(100.0))

# attention dtype: "f32" (f32r matmuls, fp32 probs) or "bf16"
ATTN_DT = os.environ.get("KERNEL_ATTN_DT", "bf16")

LAST_EXEC_NS = None
_COMPILED = {}
Exp = mybir.ActivationFunctionType.Exp
Ln = mybir.ActivationFunctionType.Ln


def _build(attn_dt: str):
    bf = attn_dt != "f32"
    AD = BF16 if bf else F32

    def mmcast(ap):
        # matmul operand dtype for the attention matmuls
        return ap if bf else ap.bitcast(F32R)

    nc = bacc.Bacc("TRN2", target_bir_lowering=False, debug=False,
                   num_devices=N_CORES)

    xT_ap = nc.dram_tensor("xT", [C, Lq], F32, kind="ExternalInput").ap()
    yT_ap = nc.dram_tensor("yT", [C, Ly], F32, kind="ExternalInput").ap()
    wq_ap = nc.dram_tensor("wq", [C, CHL], F32, kind="ExternalInput").ap()
    wk_ap = nc.dram_tensor("wk", [C, CHL], F32, kind="ExternalInput").ap()
    wv_ap = nc.dram_tensor("wv", [C, CHL], F32, kind="ExternalInput").ap()
    wo_ap = nc.dram_tensor("wo", [CHL, C], F32, kind="ExternalInput").ap()
    bq_ap = nc.dram_tensor("bq4", [128, TCH], F32, kind="ExternalInput").ap()
    is2_ap = nc.dram_tensor("invs2", [128, TCH], F32, kind="ExternalInput").ap()
    eb_ap = nc.dram_tensor("ebT", [Ly, Lq], AD, kind="ExternalInput").ap()
    out_ap = nc.dram_tensor("outT", [C, Lq], F32, kind="ExternalOutput").ap()

    xT_d = xT_ap.rearrange("(k p) i -> p k i", p=128)
    yT_d = yT_ap.rearrange("(k p) j -> p k j", p=128)
    wq_d = wq_ap.rearrange("(k p) n -> p k n", p=128)
    wk_d = wk_ap.rearrange("(k p) n -> p k n", p=128)
    wv_d = wv_ap.rearrange("(k p) n -> p k n", p=128)
    wo_d = wo_ap.rearrange("(k p) n -> p k n", p=128)
    eb_d = eb_ap.rearrange("(jt p) i -> p jt i", p=128)
    out_d = out_ap.rearrange("(ct p) i -> p ct i", p=128)

    with tile.TileContext(nc) as tc:
        with tc.tile_pool(name="persist", bufs=1) as pers, \
             tc.tile_pool(name="qn_p", bufs=1) as qn_p, \
             tc.tile_pool(name="kn_p", bufs=1) as kn_p, \
             tc.tile_pool(name="v_p", bufs=1) as v_p:

            # block-diagonal parity selector: sel.T @ sq sums each 64-row
            # head block and replicates the sums over that block's rows
            sel_f = pers.tile([128, 128], F32)
            nc.gpsimd.memset(sel_f[:], 0.0)
            nc.gpsimd.memset(sel_f[0:64, 0:64], 1.0)
            nc.gpsimd.memset(sel_f[64:128, 64:128], 1.0)
            sel_r = pers.tile([128, 128], F32)
            nc.vector.tensor_copy(sel_r[:].bitcast(F32R), sel_f[:])
            ones_a = pers.tile([128, 8], AD)
            nc.gpsimd.memset(ones_a[:], 1.0)
            bq_sb = pers.tile([128, TCH], F32)
            nc.sync.dma_start(bq_sb[:], bq_ap[:])
            is2_sb = pers.tile([128, TCH], F32)
            nc.sync.dma_start(is2_sb[:], is2_ap[:])

            qn_sb = qn_p.tile([128, TCH, Lq], AD)     # qn^T
            kn_sb = kn_p.tile([128, TCH, Ly], AD)     # kn^T
            v_sb = v_p.tile([128, NJT, HL * 65], AD)  # v (+ ones col per head)

            # yT/wv space is reserved up front; their DMAs are emitted after
            # the q-phase loads so x/wq win the DMA queues first.
            with tc.tile_pool(name="yT_p", bufs=1) as yT_p, \
                 tc.tile_pool(name="wv_p", bufs=1) as wv_p:
                yT_sb = yT_p.tile([128, KT, Ly], F32)
                wv_sb = wv_p.tile([128, KT, CHL], F32)

                # -------------- Q projection + l2norm(+scale) --------------
                with tc.tile_pool(name="xT_p", bufs=1) as xT_p, \
                     tc.tile_pool(name="wq_p", bufs=1) as wq_p, \
                     tc.tile_pool(name="qtmp", bufs=2) as qtmp, \
                     tc.tile_pool(name="qnrm", bufs=1) as qnrm, \
                     tc.tile_pool(name="psq", bufs=4, space="PSUM") as psq, \
                     tc.tile_pool(name="psqs", bufs=2, space="PSUM") as psqs:
                    xT_sb = xT_p.tile([128, KT, Lq], F32)
                    for k in range(KT):
                        nc.sync.dma_start(xT_sb[:, k, :].bitcast(F32R),
                                          xT_d[:, k, :].bitcast(F32R))
                    wq_sb = wq_p.tile([128, KT, CHL], F32)
                    nc.sync.dma_start(wq_sb[:].bitcast(F32R),
                                      wq_d[:].bitcast(F32R))
                    for k in range(KT):
                        nc.sync.dma_start(yT_sb[:, k, :].bitcast(F32R),
                                          yT_d[:, k, :].bitcast(F32R))
                    nc.sync.dma_start(wv_sb[:].bitcast(F32R),
                                      wv_d[:].bitcast(F32R))

                    for t in range(TCH):
                        qraw = qtmp.tile([128, Lq], F32, tag="qraw")
                        for ic in range(NIC):
                            ps = psq.tile([128, 512], F32)
                            for k in range(KT):
                                nc.tensor.matmul(
                                    ps[:],
                                    wq_sb[:, k, t * 128:(t + 1) * 128].bitcast(F32R),
                                    xT_sb[:, k, ic * 512:(ic + 1) * 512].bitcast(F32R),
                                    start=(k == 0), stop=(k == KT - 1))
                            nc.vector.tensor_scalar_add(
                                qraw[:, ic * 512:(ic + 1) * 512], ps[:],
                                bq_sb[:, t:t + 1])
                        sq = qnrm.tile([128, Lq], F32, tag="sq")
                        nc.vector.tensor_mul(sq[:].bitcast(F32R), qraw[:], qraw[:])
                        rs = qnrm.tile([128, Lq], F32, tag="rs")
                        for ic in range(NIC):
                            ssq = psqs.tile([128, 512], F32)
                            nc.tensor.matmul(
                                ssq[:], sel_r[:].bitcast(F32R),
                                sq[:, ic * 512:(ic + 1) * 512].bitcast(F32R),
                                start=True, stop=True)
                            # s_h/|q| = exp(-0.5*ln(sumsq/s_h^2))
                            nc.scalar.activation(
                                rs[:, ic * 512:(ic + 1) * 512], ssq[:], Ln,
                                scale=is2_sb[:, t:t + 1])
                        nc.scalar.activation(rs[:], rs[:], Exp, scale=-0.5)
                        nc.vector.tensor_mul(mmcast(qn_sb[:, t, :]),
                                             qraw[:], rs[:])

                # ------- K projection + l2norm, V proj interleaved -------
                # v matmuls are emitted between k tiles so the PE stays busy
                # while the k-norm DVE/ACT chain drains.
                with tc.tile_pool(name="wk_p", bufs=1) as wk_p, \
                     tc.tile_pool(name="ktmp", bufs=1) as ktmp, \
                     tc.tile_pool(name="psk", bufs=2, space="PSUM") as psk, \
                     tc.tile_pool(name="psv", bufs=2, space="PSUM") as psv, \
                     tc.tile_pool(name="psks", bufs=2, space="PSUM") as psks:
                    wk_sb = wk_p.tile([128, KT, CHL], F32)
                    nc.sync.dma_start(wk_sb[:].bitcast(F32R),
                                      wk_d[:].bitcast(F32R))
                    for t in range(TCH):
                        kraw = ktmp.tile([128, Ly], F32, tag="kraw", bufs=2)
                        for jc in range(Ly // 512):
                            ps = psk.tile([128, 512], F32, tag="kps")
                            for k in range(KT):
                                nc.tensor.matmul(
                                    ps[:],
                                    wk_sb[:, k, t * 128:(t + 1) * 128].bitcast(F32R),
                                    yT_sb[:, k, jc * 512:(jc + 1) * 512].bitcast(F32R),
                                    start=(k == 0), stop=(k == KT - 1))
                            nc.vector.tensor_copy(
                                kraw[:, jc * 512:(jc + 1) * 512], ps[:])
                        sqk = ktmp.tile([128, Ly], F32, tag="sqk")
                        nc.vector.tensor_mul(sqk[:].bitcast(F32R), kraw[:], kraw[:])
                        rsk = ktmp.tile([128, Ly], F32, tag="rsk")
                        for jc in range(Ly // 512):
                            ssq = psks.tile([128, 512], F32)
                            nc.tensor.matmul(
                                ssq[:], sel_r[:].bitcast(F32R),
                                sqk[:, jc * 512:(jc + 1) * 512].bitcast(F32R),
                                start=True, stop=True)
                            nc.scalar.activation(
                                rsk[:, jc * 512:(jc + 1) * 512], ssq[:], Ln)
                        nc.scalar.activation(rsk[:], rsk[:], Exp, scale=-0.5)
                        nc.vector.tensor_mul(mmcast(kn_sb[:, t, :]),
                                             kraw[:], rsk[:])

                        # v projection for jt = 4t .. 4t+3
                        for jt in range(4 * t, 4 * t + 4):
                            ps = psv.tile([128, 512], F32, tag="vps")
                            for k in range(KT):
                                nc.tensor.matmul(
                                    ps[:],
                                    yT_sb[:, k, jt * 128:(jt + 1) * 128].bitcast(F32R),
                                    wv_sb[:, k, :].bitcast(F32R),
                                    start=(k == 0), stop=(k == KT - 1))
                            vslot = v_sb[:, jt, :].rearrange(
                                "p (h e) -> p h e", e=65)
                            nc.vector.tensor_copy(
                                mmcast(vslot[:, :, 0:64]),
                                ps[:].rearrange("p (h e) -> p h e", e=64))
                            nc.vector.tensor_copy(mmcast(vslot[:, :, 64:65]),
                                                  ones_a[:, 0:HL].unsqueeze(2))

            # ---------------- attention + output projection ----------------
            with tc.tile_pool(name="wo_p", bufs=1) as wo_p, \
                 tc.tile_pool(name="oT_p", bufs=1) as oT_p, \
                 tc.tile_pool(name="eb_p", bufs=17) as eb_p, \
                 tc.tile_pool(name="pp_p", bufs=3) as pp_p, \
                 tc.tile_pool(name="nrm_p", bufs=2) as nrm_p, \
                 tc.tile_pool(name="ost_p", bufs=2) as ost_p, \
                 tc.tile_pool(name="pss", bufs=2, space="PSUM") as pss, \
                 tc.tile_pool(name="pso", bufs=3, space="PSUM") as pso, \
                 tc.tile_pool(name="psf", bufs=1, space="PSUM") as psf:
                wo_sb = wo_p.tile([128, TCH, C], F32)
                nc.sync.dma_start(wo_sb[:].bitcast(F32R), wo_d[:].bitcast(F32R))
                oT_sb = oT_p.tile([128, TCH, Lq], F32)

                for ic in range(NIC):
                    ebs = []
                    for jt in range(NJT):
                        if bf:
                            # duplicated halves so the prob multiply is a
                            # plain step-1 2D op (DVE 2x bf16 mode)
                            ebt = eb_p.tile([128, 1024], AD, tag="eb",
                                            name=f"eb{ic}_{jt}")
                            nc.sync.dma_start(
                                ebt[:, 0:512],
                                eb_d[:, jt, ic * 512:(ic + 1) * 512])
                            nc.sync.dma_start(
                                ebt[:, 512:1024],
                                eb_d[:, jt, ic * 512:(ic + 1) * 512])
                        else:
                            ebt = eb_p.tile([128, 512], AD, tag="eb",
                                            name=f"eb{ic}_{jt}")
                            nc.sync.dma_start(
                                ebt[:], eb_d[:, jt, ic * 512:(ic + 1) * 512])
                        ebs.append(ebt)
                    oas = []
                    srows = []
                    for hp in range(TCH):
                        opsums = [pso.tile([65, 512], F32, tag="opsum",
                                           name=f"opsum{_p}")
                                  for _p in range(2)]
                        for jt in range(NJT):
                            s2 = pss.tile([128, 1024], F32, tag="s")
                            for p in range(2):
                                nc.tensor.matmul(
                                    s2[:, p * 512:(p + 1) * 512],
                                    mmcast(kn_sb[p * 64:p * 64 + 64, hp,
                                                 jt * 128:(jt + 1) * 128]),
                                    mmcast(qn_sb[p * 64:p * 64 + 64, hp,
                                                 ic * 512:(ic + 1) * 512]),
                                    start=True, stop=True,
                                    tile_position=(p * 64, 0) if bf else None)
                            p0 = pp_p.tile([128, 1024], AD, tag="p0")
                            nc.scalar.activation(p0[:], s2[:], Exp)
                            pt = pp_p.tile([128, 1024], AD, tag="pt")
                            if bf:
                                nc.vector.tensor_mul(pt[:], p0[:], ebs[jt][:])
                            else:
                                nc.vector.tensor_mul(
                                    mmcast(pt[:].rearrange(
                                        "q (p i) -> q p i", p=2)),
                                    p0[:].rearrange("q (p i) -> q p i", p=2),
                                    ebs[jt][:].unsqueeze(1).broadcast_to(
                                        [128, 2, 512]))
                            for p in range(2):
                                h = 2 * hp + p
                                nc.tensor.matmul(
                                    opsums[p][:],
                                    mmcast(v_sb[:, jt, h * 65:h * 65 + 65]),
                                    mmcast(pt[:, p * 512:(p + 1) * 512]),
                                    start=(jt == 0), stop=(jt == NJT - 1))
                        for p in range(2):
                            oa = nrm_p.tile([65, 512], F32, tag="oa", bufs=9,
                                            name=f"oa{hp}_{p}")
                            nc.vector.tensor_copy(oa[:], opsums[p][:])
                            srow = nrm_p.tile([1, 512], F32, tag="srow",
                                              bufs=9, name=f"srow{hp}_{p}")
                            nc.vector.tensor_copy(srow[:], opsums[p][64:65, :])
                            oas.append(oa)
                            srows.append(srow)
                    # deferred denominators: 1/x = exp(-ln(x)); the Ln calls
                    # run adjacent, then all Exp, so the ACT spline tables
                    # load only ~2x per i-chunk instead of per head
                    with tc.tile_critical():
                        for srow in srows:
                            nc.scalar.activation(srow[:], srow[:], Ln)
                        for srow in srows:
                            nc.scalar.activation(srow[:], srow[:], Exp,
                                                 scale=-1.0)
                    for i8 in range(2 * TCH):
                        hp, p = i8 // 2, i8 % 2
                        bct = nrm_p.tile([64, 512], F32, tag="bct",
                                         name=f"bct{i8}")
                        nc.gpsimd.partition_broadcast(bct[:], srows[i8][0:1, :])
                        nc.vector.tensor_mul(
                            oT_sb[p * 64:p * 64 + 64, hp,
                                  ic * 512:(ic + 1) * 512].bitcast(F32R),
                            oas[i8][0:64, :], bct[:])

                    # output projection for this i-chunk
                    for ct in range(C // 128):
                        ps = psf.tile([128, 512], F32, tag="fout")
                        for k in range(TCH):
                            nc.tensor.matmul(
                                ps[:],
                                wo_sb[:, k, ct * 128:(ct + 1) * 128].bitcast(F32R),
                                oT_sb[:, k, ic * 512:(ic + 1) * 512].bitcast(F32R),
                                start=(k == 0), stop=(k == TCH - 1))
                        st = ost_p.tile([128, 512], F32, tag="ost")
                        nc.vector.tensor_copy(st[:], ps[:])
                        nc.sync.dma_start(
                            out_d[:, ct, ic * 512:(ic + 1) * 512], st[:])

    nc.compile()
    return nc


def _get_compiled(attn_dt: str):
    if attn_dt not in _COMPILED:
        _COMPILED[attn_dt] = _build(attn_dt)
    return _COMPILED[attn_dt]


def kernel(x, y, attn_bias, Wq, bq, Wk, Wv, Wo, bo, scale_mul_log):
    global LAST_EXEC_NS
    attn_dt = ATTN_DT
    x = np.asarray(x, dtype=np.float32)
    y = np.asarray(y, dtype=np.float32)
    attn_bias = np.asarray(attn_bias, dtype=np.float32)
    Wq = np.asarray(Wq, dtype=np.float32)
    bq = np.asarray(bq, dtype=np.float32)
    Wk = np.asarray(Wk, dtype=np.float32)
    Wv = np.asarray(Wv, dtype=np.float32)
    Wo = np.asarray(Wo, dtype=np.float32)
    bo = np.asarray(bo, dtype=np.float32)
    scale_mul_log = np.asarray(scale_mul_log, dtype=np.float32)

    nc = _get_compiled(attn_dt)

    scale = np.exp(np.minimum(scale_mul_log.reshape(H_TOT), MAX_SCALE_MUL))
    ebT = np.exp(attn_bias.T)
    ebT = np.ascontiguousarray(
        ebT.astype(np.float32 if attn_dt == "f32" else ml_dtypes.bfloat16))

    xTs = [np.ascontiguousarray(x[b].T) for b in range(B)]
    yTs = [np.ascontiguousarray(y[b].T) for b in range(B)]

    in_maps = []
    for c in range(N_CORES):
        b, g = c // 2, c % 2
        sl = slice(g * CHL, (g + 1) * CHL)
        s_loc = scale[g * HL:(g + 1) * HL]       # 8 local heads
        inv2 = 1.0 / (s_loc * s_loc)
        # invs2[p, t] = 1/s^2 of head (2t + p//64)
        invs2 = np.empty((128, TCH), dtype=np.float32)
        for t in range(TCH):
            invs2[0:64, t] = inv2[2 * t]
            invs2[64:128, t] = inv2[2 * t + 1]
        bq4 = np.ascontiguousarray(bq[sl].reshape(TCH, 128).T)
        in_maps.append({
            "xT": xTs[b],
            "yT": yTs[b],
            "wq": np.ascontiguousarray(Wq[:, sl]),
            "wk": np.ascontiguousarray(Wk[:, sl]),
            "wv": np.ascontiguousarray(Wv[:, sl]),
            "wo": np.ascontiguousarray(Wo[sl, :]),
            "bq4": bq4,
            "invs2": invs2,
            "ebT": ebT,
        })

    trace = os.environ.get("KERNEL_TRACE", "0") == "1"
    res = run_bass_kernel_spmd(nc, in_maps, core_ids=list(range(N_CORES)),
                               trace=trace)
    LAST_EXEC_NS = res.exec_time_ns

    out = np.empty((B, Lq, C), dtype=np.float32)
    for b in range(B):
        out[b] = res.results[2 * b]["outT"].T
        out[b] += res.results[2 * b + 1]["outT"].T
    out += bo
    return out


# revision 28
# speedup vs baseline: 1.5848x; 1.1256x over previous
"""Cross-attention (cosine/l2-normalized, biased softmax) on 8 TRN2 NeuronCores.

Sharding: core c handles batch b = c//2 and head group g = c%2 (8 of 16 heads,
i.e. a 512-wide slice of the QKV projections / Wo rows).  Each core computes a
partial output (its heads' contribution through Wo); the host sums the two
partials per batch and adds bo.

All tensors are kept transposed on chip (channels on partitions):
  qT/kT = (Wx)^T computed as lhsT=W, rhs=x^T; v in natural [j, ch] layout via
  lhsT=y^T.  Scores are computed transposed S^T[j, i] (lhsT = kn^T slice,
  rhs = qn^T slice, K = 64), softmax runs unnormalized as exp(S)*exp(bias)
  with the denominator obtained from an extra all-ones column appended to V,
  and the division happens after the PV matmul (partition_broadcast + mul).
L2-norm denominators use a block-diagonal selector matmul (K=128) for the
per-head sum of squares and exp(-0.5*ln(x)) on the scalar engine (the DVE
reciprocal op is ~5x an ACT pass; ACT Rsqrt is disallowed).

Matmuls run in float32r (rounded fp32, ~1.2e-4 relative); with
KERNEL_ATTN_DT=bf16 the attention matmuls (scores / PV) and the probability
tensor run in bf16 instead (f32r K=64 matmuls run at ~3.4 cyc/row on HW,
bf16 at ~1.5).
"""

import os
import numpy as np
import ml_dtypes

import concourse.bass as bass
import concourse.tile as tile
from concourse import bacc, mybir
from concourse.bass_utils import run_bass_kernel_spmd

F32 = mybir.dt.float32
F32R = mybir.dt.float32r
BF16 = mybir.dt.bfloat16

B, Lq, Ly, C = 4, 1024, 2048, 1024
H_TOT, D = 16, 64
HL = 8           # heads per core
CHL = HL * D     # 512 channels per core
TCH = CHL // 128  # 4 channel tiles (2 heads each)
KT = C // 128     # 8 contraction tiles for the projections
NJT = Ly // 128   # 16 j tiles
NIC = Lq // 512   # 2 i chunks
N_CORES = 8
MAX_SCALE_MUL = float(np.log(100.0))

# attention dtype: "f32" (f32r matmuls, fp32 probs) or "bf16"
ATTN_DT = os.environ.get("KERNEL_ATTN_DT", "bf16")

LAST_EXEC_NS = None
_COMPILED = {}
Exp = mybir.ActivationFunctionType.Exp
Ln = mybir.ActivationFunctionType.Ln

_ACT_TABLES_INSTALLED = False


def _install_act_tables():
    """Point both bacc and walrus at an act_info.json with the combined
    ln+exp function set first, so Ln/Exp alternation (softmax denominators,
    l2-norm rsqrt via exp(-0.5 ln x)) stops thrashing the ACT spline table
    (~1.3 us per reload).  Selection is first-match over the set list."""
    global _ACT_TABLES_INSTALLED
    if _ACT_TABLES_INSTALLED:
        return
    import json
    import shutil
    import tempfile
    import concourse.hw_specs as hw_specs
    import concourse.bacc as bacc_mod
    try:
        from neuronxcc.driver.Job import Job
        from neuronxcc.driver.jobs.support.FindActInfo import findActInfoFile
        src = findActInfoFile(Job.getPackageDir(), "gen3")
    except Exception:
        return
    dst = os.path.join(tempfile.mkdtemp(prefix="actpwp"), "pwp")
    shutil.copytree(os.path.dirname(src), dst)
    info_path = os.path.join(dst, "act_info.json")
    with open(info_path) as f:
        info = json.load(f)
    key = "natural_log_exp_and_others"
    info["act_func_sets"].sort(key=lambda s: 0 if s["name"] == key else 1)
    with open(info_path, "w") as f:
        json.dump(info, f)
    os.environ["BASS_ACT_ROOT_JSON_PATH"] = info_path

    orig = hw_specs.get_activation_tables

    def reordered(arch):
        d = orig(arch)
        if key not in d:
            return d
        out = {key: d[key]}
        out.update((k, v) for k, v in d.items() if k != key)
        return out

    hw_specs.get_activation_tables = reordered
    bacc_mod.get_activation_tables = reordered
    _ACT_TABLES_INSTALLED = True


def _build(attn_dt: str):
    _install_act_tables()
    bf = attn_dt != "f32"
    AD = BF16 if bf else F32

    def mmcast(ap):
        # matmul operand dtype for the attention matmuls
        return ap if bf else ap.bitcast(F32R)

    nc = bacc.Bacc("TRN2", target_bir_lowering=False, debug=False,
                   num_devices=N_CORES)

    xT_ap = nc.dram_tensor("xT", [C, Lq], F32, kind="ExternalInput").ap()
    yT_ap = nc.dram_tensor("yT", [C, Ly], F32, kind="ExternalInput").ap()
    wq_ap = nc.dram_tensor("wq", [C, CHL], F32, kind="ExternalInput").ap()
    wk_ap = nc.dram_tensor("wk", [C, CHL], F32, kind="ExternalInput").ap()
    wv_ap = nc.dram_tensor("wv", [C, CHL], F32, kind="ExternalInput").ap()
    wo_ap = nc.dram_tensor("wo", [CHL, C], F32, kind="ExternalInput").ap()
    bq_ap = nc.dram_tensor("bq4", [128, TCH], F32, kind="ExternalInput").ap()
    is2_ap = nc.dram_tensor("invs2", [128, TCH], F32, kind="ExternalInput").ap()
    eb_ap = nc.dram_tensor("ebT", [Ly, Lq], AD, kind="ExternalInput").ap()
    out_ap = nc.dram_tensor("outT", [C, Lq], F32, kind="ExternalOutput").ap()

    xT_d = xT_ap.rearrange("(k p) i -> p k i", p=128)
    yT_d = yT_ap.rearrange("(k p) j -> p k j", p=128)
    wq_d = wq_ap.rearrange("(k p) n -> p k n", p=128)
    wk_d = wk_ap.rearrange("(k p) n -> p k n", p=128)
    wv_d = wv_ap.rearrange("(k p) n -> p k n", p=128)
    wo_d = wo_ap.rearrange("(k p) n -> p k n", p=128)
    eb_d = eb_ap.rearrange("(jt p) i -> p jt i", p=128)
    out_d = out_ap.rearrange("(ct p) i -> p ct i", p=128)

    with tile.TileContext(nc) as tc:
        with tc.tile_pool(name="persist", bufs=1) as pers, \
             tc.tile_pool(name="qn_p", bufs=1) as qn_p, \
             tc.tile_pool(name="kn_p", bufs=1) as kn_p, \
             tc.tile_pool(name="v_p", bufs=1) as v_p:

            # block-diagonal parity selector: sel.T @ sq sums each 64-row
            # head block and replicates the sums over that block's rows
            sel_f = pers.tile([128, 128], F32)
            nc.gpsimd.memset(sel_f[:], 0.0)
            nc.gpsimd.memset(sel_f[0:64, 0:64], 1.0)
            nc.gpsimd.memset(sel_f[64:128, 64:128], 1.0)
            sel_r = pers.tile([128, 128], F32)
            nc.vector.tensor_copy(sel_r[:].bitcast(F32R), sel_f[:])
            ones_a = pers.tile([128, 8], AD)
            nc.gpsimd.memset(ones_a[:], 1.0)
            bq_sb = pers.tile([128, TCH], F32)
            nc.sync.dma_start(bq_sb[:], bq_ap[:])
            is2_sb = pers.tile([128, TCH], F32)
            nc.sync.dma_start(is2_sb[:], is2_ap[:])

            qn_sb = qn_p.tile([128, TCH, Lq], AD)     # qn^T
            kn_sb = kn_p.tile([128, TCH, Ly], AD)     # kn^T
            v_sb = v_p.tile([128, NJT, HL * 65], AD)  # v (+ ones col per head)

            # yT/wv space is reserved up front; their DMAs are emitted after
            # the q-phase loads so x/wq win the DMA queues first.
            with tc.tile_pool(name="yT_p", bufs=1) as yT_p, \
                 tc.tile_pool(name="wv_p", bufs=1) as wv_p:
                yT_sb = yT_p.tile([128, KT, Ly], F32)
                wv_sb = wv_p.tile([128, KT, CHL], F32)

                # -------------- Q projection + l2norm(+scale) --------------
                with tc.tile_pool(name="xT_p", bufs=1) as xT_p, \
                     tc.tile_pool(name="wq_p", bufs=1) as wq_p, \
                     tc.tile_pool(name="qtmp", bufs=2) as qtmp, \
                     tc.tile_pool(name="qnrm", bufs=1) as qnrm, \
                     tc.tile_pool(name="psq", bufs=4, space="PSUM") as psq, \
                     tc.tile_pool(name="psqs", bufs=2, space="PSUM") as psqs:
                    xT_sb = xT_p.tile([128, KT, Lq], F32)
                    for k in range(KT):
                        nc.sync.dma_start(xT_sb[:, k, :].bitcast(F32R),
                                          xT_d[:, k, :].bitcast(F32R))
                    wq_sb = wq_p.tile([128, KT, CHL], F32)
                    nc.sync.dma_start(wq_sb[:].bitcast(F32R),
                                      wq_d[:].bitcast(F32R))
                    for k in range(KT):
                        nc.sync.dma_start(yT_sb[:, k, :].bitcast(F32R),
                                          yT_d[:, k, :].bitcast(F32R))
                    nc.sync.dma_start(wv_sb[:].bitcast(F32R),
                                      wv_d[:].bitcast(F32R))

                    for t in range(TCH):
                        qraw = qtmp.tile([128, Lq], F32, tag="qraw")
                        for ic in range(NIC):
                            ps = psq.tile([128, 512], F32)
                            for k in range(KT):
                                nc.tensor.matmul(
                                    ps[:],
                                    wq_sb[:, k, t * 128:(t + 1) * 128].bitcast(F32R),
                                    xT_sb[:, k, ic * 512:(ic + 1) * 512].bitcast(F32R),
                                    start=(k == 0), stop=(k == KT - 1))
                            nc.vector.tensor_scalar_add(
                                qraw[:, ic * 512:(ic + 1) * 512], ps[:],
                                bq_sb[:, t:t + 1])
                        sq = qnrm.tile([128, Lq], F32, tag="sq")
                        nc.vector.tensor_mul(sq[:].bitcast(F32R), qraw[:], qraw[:])
                        rs = qnrm.tile([128, Lq], F32, tag="rs")
                        for ic in range(NIC):
                            ssq = psqs.tile([128, 512], F32)
                            nc.tensor.matmul(
                                ssq[:], sel_r[:].bitcast(F32R),
                                sq[:, ic * 512:(ic + 1) * 512].bitcast(F32R),
                                start=True, stop=True)
                            # s_h/|q| = exp(-0.5*ln(sumsq/s_h^2))
                            nc.scalar.activation(
                                rs[:, ic * 512:(ic + 1) * 512], ssq[:], Ln,
                                scale=is2_sb[:, t:t + 1])
                        nc.scalar.activation(rs[:], rs[:], Exp, scale=-0.5)
                        nc.vector.tensor_mul(mmcast(qn_sb[:, t, :]),
                                             qraw[:], rs[:])

                # ------- K projection + l2norm, V proj interleaved -------
                # v matmuls are emitted between k tiles so the PE stays busy
                # while the k-norm DVE/ACT chain drains.
                with tc.tile_pool(name="wk_p", bufs=1) as wk_p, \
                     tc.tile_pool(name="ktmp", bufs=1) as ktmp, \
                     tc.tile_pool(name="psk", bufs=2, space="PSUM") as psk, \
                     tc.tile_pool(name="psv", bufs=2, space="PSUM") as psv, \
                     tc.tile_pool(name="psks", bufs=2, space="PSUM") as psks:
                    wk_sb = wk_p.tile([128, KT, CHL], F32)
                    nc.sync.dma_start(wk_sb[:].bitcast(F32R),
                                      wk_d[:].bitcast(F32R))
                    for t in range(TCH):
                        kraw = ktmp.tile([128, Ly], F32, tag="kraw", bufs=2)
                        for jc in range(Ly // 512):
                            ps = psk.tile([128, 512], F32, tag="kps")
                            for k in range(KT):
                                nc.tensor.matmul(
                                    ps[:],
                                    wk_sb[:, k, t * 128:(t + 1) * 128].bitcast(F32R),
                                    yT_sb[:, k, jc * 512:(jc + 1) * 512].bitcast(F32R),
                                    start=(k == 0), stop=(k == KT - 1))
                            nc.vector.tensor_copy(
                                kraw[:, jc * 512:(jc + 1) * 512], ps[:])
                        sqk = ktmp.tile([128, Ly], F32, tag="sqk")
                        nc.vector.tensor_mul(sqk[:].bitcast(F32R), kraw[:], kraw[:])
                        rsk = ktmp.tile([128, Ly], F32, tag="rsk")
                        for jc in range(Ly // 512):
                            ssq = psks.tile([128, 512], F32)
                            nc.tensor.matmul(
                                ssq[:], sel_r[:].bitcast(F32R),
                                sqk[:, jc * 512:(jc + 1) * 512].bitcast(F32R),
                                start=True, stop=True)
                            nc.scalar.activation(
                                rsk[:, jc * 512:(jc + 1) * 512], ssq[:], Ln)
                        nc.scalar.activation(rsk[:], rsk[:], Exp, scale=-0.5)
                        nc.vector.tensor_mul(mmcast(kn_sb[:, t, :]),
                                             kraw[:], rsk[:])

                        # v projection for jt = 4t .. 4t+3
                        for jt in range(4 * t, 4 * t + 4):
                            ps = psv.tile([128, 512], F32, tag="vps")
                            for k in range(KT):
                                nc.tensor.matmul(
                                    ps[:],
                                    yT_sb[:, k, jt * 128:(jt + 1) * 128].bitcast(F32R),
                                    wv_sb[:, k, :].bitcast(F32R),
                                    start=(k == 0), stop=(k == KT - 1))
                            vslot = v_sb[:, jt, :].rearrange(
                                "p (h e) -> p h e", e=65)
                            nc.vector.tensor_copy(
                                mmcast(vslot[:, :, 0:64]),
                                ps[:].rearrange("p (h e) -> p h e", e=64))
                            nc.vector.tensor_copy(mmcast(vslot[:, :, 64:65]),
                                                  ones_a[:, 0:HL].unsqueeze(2))

            # ---------------- attention + output projection ----------------
            with tc.tile_pool(name="wo_p", bufs=1) as wo_p, \
                 tc.tile_pool(name="oT_p", bufs=1) as oT_p, \
                 tc.tile_pool(name="eb_p", bufs=17) as eb_p, \
                 tc.tile_pool(name="pp_p", bufs=3) as pp_p, \
                 tc.tile_pool(name="nrm_p", bufs=2) as nrm_p, \
                 tc.tile_pool(name="ost_p", bufs=2) as ost_p, \
                 tc.tile_pool(name="pss", bufs=2, space="PSUM") as pss, \
                 tc.tile_pool(name="pso", bufs=3, space="PSUM") as pso, \
                 tc.tile_pool(name="psf", bufs=1, space="PSUM") as psf:
                wo_sb = wo_p.tile([128, TCH, C], F32)
                nc.sync.dma_start(wo_sb[:].bitcast(F32R), wo_d[:].bitcast(F32R))
                oT_sb = oT_p.tile([128, TCH, Lq], F32)

                for ic in range(NIC):
                    ebs = []
                    for jt in range(NJT):
                        if bf:
                            # duplicated halves so the prob multiply is a
                            # plain step-1 2D op (DVE 2x bf16 mode)
                            ebt = eb_p.tile([128, 1024], AD, tag="eb",
                                            name=f"eb{ic}_{jt}")
                            nc.sync.dma_start(
                                ebt[:, 0:512],
                                eb_d[:, jt, ic * 512:(ic + 1) * 512])
                            nc.sync.dma_start(
                                ebt[:, 512:1024],
                                eb_d[:, jt, ic * 512:(ic + 1) * 512])
                        else:
                            ebt = eb_p.tile([128, 512], AD, tag="eb",
                                            name=f"eb{ic}_{jt}")
                            nc.sync.dma_start(
                                ebt[:], eb_d[:, jt, ic * 512:(ic + 1) * 512])
                        ebs.append(ebt)
                    oas = []
                    srows = []
                    for hp in range(TCH):
                        opsums = [pso.tile([65, 512], F32, tag="opsum",
                                           name=f"opsum{_p}")
                                  for _p in range(2)]
                        # software pipeline: scores(jt) are emitted before
                        # exp/mul/PV of jt-1 so the two K=64 score matmuls
                        # keep adjacent scheduler priority (they execute
                        # concurrently on disjoint PE row groups)
                        s2s = [None] * NJT

                        def tail(jt):
                            p0 = pp_p.tile([128, 1024], AD, tag="p0",
                                           name=f"p0_{jt}")
                            nc.scalar.activation(p0[:], s2s[jt][:], Exp)
                            pt = pp_p.tile([128, 1024], AD, tag="pt",
                                           name=f"pt_{jt}")
                            if bf:
                                nc.vector.tensor_mul(pt[:], p0[:], ebs[jt][:])
                            else:
                                nc.vector.tensor_mul(
                                    mmcast(pt[:].rearrange(
                                        "q (p i) -> q p i", p=2)),
                                    p0[:].rearrange("q (p i) -> q p i", p=2),
                                    ebs[jt][:].unsqueeze(1).broadcast_to(
                                        [128, 2, 512]))
                            for p in range(2):
                                h = 2 * hp + p
                                nc.tensor.matmul(
                                    opsums[p][:],
                                    mmcast(v_sb[:, jt, h * 65:h * 65 + 65]),
                                    mmcast(pt[:, p * 512:(p + 1) * 512]),
                                    start=(jt == 0), stop=(jt == NJT - 1))

                        for jt in range(NJT):
                            s2s[jt] = pss.tile([128, 1024], F32, tag="s",
                                               name=f"s2_{jt}")
                            for p in range(2):
                                nc.tensor.matmul(
                                    s2s[jt][:, p * 512:(p + 1) * 512],
                                    mmcast(kn_sb[p * 64:p * 64 + 64, hp,
                                                 jt * 128:(jt + 1) * 128]),
                                    mmcast(qn_sb[p * 64:p * 64 + 64, hp,
                                                 ic * 512:(ic + 1) * 512]),
                                    start=True, stop=True,
                                    tile_position=(p * 64, 0) if bf else None)
                            if jt > 0:
                                tail(jt - 1)
                        tail(NJT - 1)
                        for p in range(2):
                            oa = nrm_p.tile([65, 512], F32, tag="oa", bufs=9,
                                            name=f"oa{hp}_{p}")
                            nc.vector.tensor_copy(oa[:], opsums[p][:])
                            srow = nrm_p.tile([1, 512], F32, tag="srow",
                                              bufs=9, name=f"srow{hp}_{p}")
                            nc.vector.tensor_copy(srow[:], opsums[p][64:65, :])
                            oas.append(oa)
                            srows.append(srow)
                    # deferred denominators: 1/x = exp(-ln(x)); the Ln calls
                    # run adjacent, then all Exp, so the ACT spline tables
                    # load only ~2x per i-chunk instead of per head
                    for srow in srows:
                        nc.scalar.activation(srow[:], srow[:], Ln)
                    for srow in srows:
                        nc.scalar.activation(srow[:], srow[:], Exp, scale=-1.0)
                    for i8 in range(2 * TCH):
                        hp, p = i8 // 2, i8 % 2
                        bct = nrm_p.tile([64, 512], F32, tag="bct",
                                         name=f"bct{i8}")
                        nc.gpsimd.partition_broadcast(bct[:], srows[i8][0:1, :])
                        nc.vector.tensor_mul(
                            oT_sb[p * 64:p * 64 + 64, hp,
                                  ic * 512:(ic + 1) * 512].bitcast(F32R),
                            oas[i8][0:64, :], bct[:])

                    # output projection for this i-chunk
                    for ct in range(C // 128):
                        ps = psf.tile([128, 512], F32, tag="fout")
                        for k in range(TCH):
                            nc.tensor.matmul(
                                ps[:],
                                wo_sb[:, k, ct * 128:(ct + 1) * 128].bitcast(F32R),
                                oT_sb[:, k, ic * 512:(ic + 1) * 512].bitcast(F32R),
                                start=(k == 0), stop=(k == TCH - 1))
                        st = ost_p.tile([128, 512], F32, tag="ost")
                        nc.vector.tensor_copy(st[:], ps[:])
                        nc.sync.dma_start(
                            out_d[:, ct, ic * 512:(ic + 1) * 512], st[:])

    nc.compile()
    return nc


def _get_compiled(attn_dt: str):
    if attn_dt not in _COMPILED:
        _COMPILED[attn_dt] = _build(attn_dt)
    return _COMPILED[attn_dt]


def kernel(x, y, attn_bias, Wq, bq, Wk, Wv, Wo, bo, scale_mul_log):
    global LAST_EXEC_NS
    attn_dt = ATTN_DT
    x = np.asarray(x, dtype=np.float32)
    y = np.asarray(y, dtype=np.float32)
    attn_bias = np.asarray(attn_bias, dtype=np.float32)
    Wq = np.asarray(Wq, dtype=np.float32)
    bq = np.asarray(bq, dtype=np.float32)
    Wk = np.asarray(Wk, dtype=np.float32)
    Wv = np.asarray(Wv, dtype=np.float32)
    Wo = np.asarray(Wo, dtype=np.float32)
    bo = np.asarray(bo, dtype=np.float32)
    scale_mul_log = np.asarray(scale_mul_log, dtype=np.float32)

    nc = _get_compiled(attn_dt)

    scale = np.exp(np.minimum(scale_mul_log.reshape(H_TOT), MAX_SCALE_MUL))
    ebT = np.exp(attn_bias.T)
    ebT = np.ascontiguousarray(
        ebT.astype(np.float32 if attn_dt == "f32" else ml_dtypes.bfloat16))

    xTs = [np.ascontiguousarray(x[b].T) for b in range(B)]
    yTs = [np.ascontiguousarray(y[b].T) for b in range(B)]

    in_maps = []
    for c in range(N_CORES):
        b, g = c // 2, c % 2
        sl = slice(g * CHL, (g + 1) * CHL)
        s_loc = scale[g * HL:(g + 1) * HL]       # 8 local heads
        inv2 = 1.0 / (s_loc * s_loc)
        # invs2[p, t] = 1/s^2 of head (2t + p//64)
        invs2 = np.empty((128, TCH), dtype=np.float32)
        for t in range(TCH):
            invs2[0:64, t] = inv2[2 * t]
            invs2[64:128, t] = inv2[2 * t + 1]
        bq4 = np.ascontiguousarray(bq[sl].reshape(TCH, 128).T)
        in_maps.append({
            "xT": xTs[b],
            "yT": yTs[b],
            "wq": np.ascontiguousarray(Wq[:, sl]),
            "wk": np.ascontiguousarray(Wk[:, sl]),
            "wv": np.ascontiguousarray(Wv[:, sl]),
            "wo": np.ascontiguousarray(Wo[sl, :]),
            "bq4": bq4,
            "invs2": invs2,
            "ebT": ebT,
        })

    trace = os.environ.get("KERNEL_TRACE", "0") == "1"
    res = run_bass_kernel_spmd(nc, in_maps, core_ids=list(range(N_CORES)),
                               trace=trace)
    LAST_EXEC_NS = res.exec_time_ns

    out = np.empty((B, Lq, C), dtype=np.float32)
    for b in range(B):
        out[b] = res.results[2 * b]["outT"].T
        out[b] += res.results[2 * b + 1]["outT"].T
    out += bo
    return out
